# revision 2
# baseline (speedup 1.0000x reference)
"""Trainium2 Bass kernel for an 8-expert MoE FFN layer (nn_MoELayer).

Reference computation (per expert e over its contiguous 1024-token chunk):
    h = gelu(x_e @ w1[e] + b1[e]);  y_e = h @ w2[e] + b2[e]

Sharding: expert parallelism — core e holds expert e's weights and its token
chunk (the gate yields equal contiguous chunks, so no all-to-all is needed).
Each core runs the same SPMD program on its own data.

Per-core kernel (T=1024 tokens, D=1024, F=4096), all matmuls in fp16 with
fp32 PSUM accumulation (215.8 ns per 512-wide matmul — the PE's warm floor;
fp32 would be 4x slower, fp8 fails the accuracy gate):
  phase 1: for each 128-wide f-tile: h^T[ft] = gelu(w1[:,ft]^T @ x^T + b1[ft])
           (f on partitions -> b1 is a per-partition ACT bias; h^T resident in SBUF)
  phase 2: for each 128-wide dm-tile: y^T[dmo] = w2[:,dmo]^T @ h^T + b2[dmo]
           (dm-tile outer -> one 2-bank PSUM accumulator at a time)
All layout transposes/repacks are done on the host so every DMA is a large
partition-contiguous stream.

Head: descriptor issue costs ~0.65us/DMA per queue and doorbell-to-data is
~2us, so the critical first tiles (w1[0..4], x chunk 0 in quarters, x chunk 1
in halves, biases) are issued in parallel across the three DMA-capable queues
(Sync, Scalar, GpSimd) right at kernel start. A short burst of dummy matmuls
on memset scratch bridges the PE from kernel start to first-data and begins
warming the HAM clock gate; the first real matmul groups then run while the
clock finishes warming. Tail: the last dm-tile accumulates in 512/256/256
column chunks so the final flush (bias-add + store) is short, with the last
two stores issued on different queues so their descriptor writes overlap.
"""

import os

import numpy as np

# The kernel executes through the axon PJRT backend; a CPU pin (e.g. set for
# a jax reference run) would break NEFF dispatch in this process.
if os.environ.get("JAX_PLATFORMS") == "cpu":
    del os.environ["JAX_PLATFORMS"]

E = 8          # experts == cores
B, S = 2, 4096
D = 1024       # d_model
F = 4096       # d_ff
T = (B * S) // E  # tokens per expert chunk = 1024
P = 128
DO = D // P    # 8  k-tiles of d_model
FT = F // P    # 32 f-tiles of d_ff
DMO = D // P   # 8  output dm-tiles
FT2 = FT // 2  # half-slab of w2 f-tiles
NCHUNK = T // 512  # 2 moving-operand chunks (PSUM bank caps matmul N at 512)
N_WARMUP_MM = 5
N_PRELOAD = 5  # w1 tiles issued during the head DMA burst

_cached = None


def _build():
    import concourse.mybir as mybir
    import concourse.tile as tile
    from concourse import bacc
    from concourse.tile_rust import add_dep_helper

    f32 = mybir.dt.float32
    f16 = mybir.dt.float16

    nc = bacc.Bacc("TRN2", target_bir_lowering=False, debug=False, num_devices=E)

    xT_d = nc.dram_tensor("xT", [NCHUNK, P, DO, 512], f16, kind="ExternalInput")
    w1_d = nc.dram_tensor("w1r", [FT, P, DO, P], f16, kind="ExternalInput")
    bc_d = nc.dram_tensor("bc", [P, FT + DMO], f32, kind="ExternalInput")
    w2_d = nc.dram_tensor("w2r", [DMO, 2, P, FT2, P], f16, kind="ExternalInput")
    yT_d = nc.dram_tensor("yT", [DMO, P, T], f32, kind="ExternalOutput")

    gelu = mybir.ActivationFunctionType.Gelu_apprx_tanh

    with tile.TileContext(nc) as tc:
        with (
            tc.tile_pool(name="xpool", bufs=1) as xpool,
            tc.tile_pool(name="hpool", bufs=1) as hpool,
            tc.tile_pool(name="wpool", bufs=2) as wpool,
            tc.tile_pool(name="cpool", bufs=1) as cpool,
            tc.tile_pool(name="ypool", bufs=2) as ypool,
            tc.tile_pool(name="psum_h", bufs=2, space="PSUM") as psum_h,
            tc.tile_pool(name="psum_y", bufs=2, space="PSUM") as psum_y,
        ):
            # ---- head: queue the critical input DMAs across all three
            # DMA-capable queues so descriptor issue (~0.65us each, serial
            # per queue) and the ~2us doorbell-to-data latency overlap.
            w1_tiles = {}
            for ft in range(N_PRELOAD):
                w1_tiles[ft] = wpool.tile(
                    [P, DO, P], f16, tag="w1", bufs=7, name="w1_sb"
                )
            # contiguous-per-partition destination: [p, c, do*512]
            xT_sb = xpool.tile([P, NCHUNK, DO * 512], f16)
            bc_sb = cpool.tile([P, FT + DMO], f32)

            QX = DO * 512 // 4  # x chunk-0 quarter (2 do-tiles)
            HX = DO * 512 // 2  # x chunk-1 half (4 do-tiles)
            xc0 = xT_d.ap()[0].rearrange("p do t -> p (do t)")
            xc1 = xT_d.ap()[1].rearrange("p do t -> p (do t)")
            # Sync queue: w1[0] first (group 1's stationary), then x0 q0/q1
            nc.sync.dma_start(w1_tiles[0][:], w1_d.ap()[0])
            nc.sync.dma_start(xT_sb[:, 0, :QX], xc0[:, :QX])
            nc.sync.dma_start(xT_sb[:, 0, QX : 2 * QX], xc0[:, QX : 2 * QX])
            nc.sync.dma_start(w1_tiles[3][:], w1_d.ap()[3])
            # Scalar queue (free until the first gelu ~10us in): x0 q2/q3,
            # w1[1], w1[4]
            nc.scalar.dma_start(xT_sb[:, 0, 2 * QX : 3 * QX], xc0[:, 2 * QX : 3 * QX])
            nc.scalar.dma_start(xT_sb[:, 0, 3 * QX :], xc0[:, 3 * QX :])
            nc.scalar.dma_start(w1_tiles[1][:], w1_d.ap()[1])
            nc.scalar.dma_start(w1_tiles[4][:], w1_d.ap()[4])
            # GpSimd (software DGE, higher latency — non-critical tiles):
            # w1[2], biases, then x chunk 1 halves (needed from group 4 on)
            nc.gpsimd.dma_start(w1_tiles[2][:], w1_d.ap()[2])
            nc.gpsimd.dma_start(bc_sb[:], bc_d.ap())
            nc.gpsimd.dma_start(xT_sb[:, 1, :HX], xc1[:, :HX])
            nc.gpsimd.dma_start(xT_sb[:, 1, HX:], xc1[:, HX:])
            b1_sb = bc_sb[:, :FT]
            b2_sb = bc_sb[:, FT:]

            # scratch for PE warmup: direct fp16 memset on the (otherwise
            # idle) vector engine — no fp32 staging, no cast
            scratch = cpool.tile([P, 512], f16)
            nc.vector.memset(scratch[:], 0.0)

            # PE warmup: dummy matmuls on scratch while the head DMAs are in
            # flight. Bridges kernel-start -> first-data and starts the HAM
            # clock-gate warming.
            for i in range(N_WARMUP_MM):
                pw = psum_y.tile([P, 512], f32, tag="py", name="pwarm")
                nc.tensor.matmul(
                    pw[:], scratch[:, :P], scratch[:], start=True, stop=True
                )

            h_sb = hpool.tile([P, FT, T], f16)

            # ---- phase 1: h^T = gelu(w1^T x^T + b1), one 128-row f-tile at a time
            # per-(ft, chunk) 1-bank PSUM tiles; the first three f-tiles run
            # chunk-0 first so the PE streams while x chunk 1 is on the wire
            def mm1_group(ph, w1_sb, c):
                mm = None
                for do in range(DO):
                    mm = nc.tensor.matmul(
                        ph[:],
                        w1_sb[:, do, :],
                        xT_sb[:, c, do * 512 : (do + 1) * 512],
                        start=(do == 0),
                        stop=(do == DO - 1),
                    )
                return mm

            def gelu_chunk(ph, ft, c):
                cs = slice(c * 512, (c + 1) * 512)
                return nc.scalar.activation(
                    h_sb[:, ft, cs], ph[:], gelu, bias=b1_sb[:, ft : ft + 1]
                )

            gelu_insts = {}
            HEAD = 3
            head_ph = {}
            for ft in range(HEAD):
                ph = psum_h.tile([P, 512], f32, tag="ph", bufs=4, name="ph")
                head_ph[ft] = ph
                mm1_group(ph, w1_tiles[ft], 0)
            for ft in range(HEAD):
                ph = head_ph[ft]
                gelu_insts[(ft, 0)] = gelu_chunk(ph, ft, 0)
                ph2 = psum_h.tile([P, 512], f32, tag="ph", bufs=4, name="ph")
                mm1_group(ph2, w1_tiles[ft], 1)
                gelu_insts[(ft, 1)] = gelu_chunk(ph2, ft, 1)

            for ft in range(HEAD, FT):
                if ft >= N_PRELOAD:
                    w1_tiles[ft] = wpool.tile(
                        [P, DO, P], f16, tag="w1", bufs=7, name="w1_sb"
                    )
                    nc.sync.dma_start(w1_tiles[ft][:], w1_d.ap()[ft])
                w1_sb = w1_tiles[ft]
                for c in range(NCHUNK):
                    ph = psum_h.tile([P, 512], f32, tag="ph", bufs=4, name="ph")
                    mm1_group(ph, w1_sb, c)
                    gelu_insts[(ft, c)] = gelu_chunk(ph, ft, c)

            # ---- phase 2: y^T[dmo] = w2[:,dmo]^T h^T + b2[dmo]
            FQ = FT // 4
            for dmo in range(DMO):
                w2_q = []
                for qq in range(4):
                    w2_sb = wpool.tile([P, FQ, P], f16, tag="w2", bufs=32, name="w2_sb")
                    dma = nc.sync.dma_start(
                        w2_sb[:],
                        w2_d.ap()[dmo, qq // 2, :, (qq % 2) * FQ : (qq % 2 + 1) * FQ],
                    )
                    if dmo == 0:
                        # keep dmo 0's prefetch out of the head's w1/xT window
                        add_dep_helper(
                            dma.ins,
                            gelu_insts[(6, 1)].ins,
                            sync=True,
                            reason="delay w2 prefetch past the kernel head",
                        )
                    w2_q.append(w2_sb)

                def y_flush(py_ap, cs, engine):
                    # bias-add in 256-col chunks so stores overlap the adds
                    y_sb = ypool.tile([P, 256], f32, tag="y", bufs=4, name="y_sb")
                    nc.vector.tensor_scalar_add(
                        y_sb[:], py_ap, b2_sb[:, dmo : dmo + 1]
                    )
                    engine.dma_start(yT_d.ap()[dmo, :, cs], y_sb[:])

                if dmo < DMO - 1:
                    py = psum_y.tile([P, T], f32, tag="py", name="py")
                    for fo in range(FT):
                        wt = w2_q[fo // FQ][:, fo % FQ, :]
                        for c in range(NCHUNK):
                            cs = slice(c * 512, (c + 1) * 512)
                            nc.tensor.matmul(
                                py[:, cs],
                                wt,
                                h_sb[:, fo, cs],
                                start=(fo == 0),
                                stop=(fo == FT - 1),
                            )
                    for c in range(4):
                        cs = slice(c * 256, (c + 1) * 256)
                        y_flush(py[:, cs], cs, nc.sync)
                else:
                    # last dm-tile: accumulate in 512/256/256-column chunks so
                    # only one short bias-add + store trails the final matmul.
                    # Separate PSUM tiles per chunk so Tile doesn't serialize
                    # one chunk's reads against the next chunk's writes. The
                    # final two stores go out on different queues so their
                    # ~0.6us descriptor writes overlap.
                    chunks = [(0, 512), (512, 256), (768, 256)]
                    for ci, (c0, cw) in enumerate(chunks):
                        py_c = psum_y.tile([P, cw], f32, tag="py", name="py_c")
                        for fo in range(FT):
                            wt = w2_q[fo // FQ][:, fo % FQ, :]
                            nc.tensor.matmul(
                                py_c[:],
                                wt,
                                h_sb[:, fo, c0 : c0 + cw],
                                start=(fo == 0),
                                stop=(fo == FT - 1),
                            )
                        for cc in range(cw // 256):
                            cs = slice(c0 + cc * 256, c0 + (cc + 1) * 256)
                            last = ci == len(chunks) - 1
                            y_flush(
                                py_c[:, cc * 256 : (cc + 1) * 256],
                                cs,
                                nc.scalar if last else nc.sync,
                            )

    nc.compile()
    return nc


def _get_nc():
    global _cached
    if _cached is None:
        _cached = _build()
    return _cached


def make_in_maps(x, w1, b1, w2, b2):
    x = np.asarray(x, dtype=np.float32)
    w1 = np.asarray(w1, dtype=np.float32)
    b1 = np.asarray(b1, dtype=np.float32)
    w2 = np.asarray(w2, dtype=np.float32)
    b2 = np.asarray(b2, dtype=np.float32)

    tokens = x.reshape(E, T, D)
    in_maps = []
    for e in range(E):
        xT = np.ascontiguousarray(
            tokens[e].reshape(NCHUNK, 512, DO, P).transpose(0, 3, 2, 1)
        ).astype(np.float16)  # [c, p, do, t']
        w1r = np.ascontiguousarray(
            w1[e].reshape(DO, P, FT, P).transpose(2, 1, 0, 3)
        ).astype(np.float16)  # [ft, p, do, j]
        bc = np.ascontiguousarray(
            np.concatenate([b1[e].reshape(FT, P).T, b2[e].reshape(DMO, P).T], axis=1)
        )  # [p, ft..dmo]
        w2r = np.ascontiguousarray(
            w2[e].reshape(2, FT2, P, DMO, P).transpose(3, 0, 2, 1, 4)
        ).astype(np.float16)  # [dmo, half, p, fo, j]
        in_maps.append({"xT": xT, "w1r": w1r, "bc": bc, "w2r": w2r})
    return in_maps


def gather_out(results):
    out = np.empty((E, T, D), dtype=np.float32)
    for e in range(E):
        yT = results[e]["yT"]  # [dmo, p, t]
        out[e] = yT.transpose(2, 0, 1).reshape(T, D)
    return out.reshape(B, S, D)


def kernel(x, w1, b1, w2, b2):
    from concourse.bass_utils import run_bass_kernel_spmd

    nc = _get_nc()
    in_maps = make_in_maps(x, w1, b1, w2, b2)
    res = run_bass_kernel_spmd(nc, in_maps, core_ids=list(range(E)))
    return gather_out(res.results)


# revision 4
# speedup vs baseline: 1.0128x; 1.0128x over previous
"""Trainium2 Bass kernel for an 8-expert MoE FFN layer (nn_MoELayer).

Reference computation (per expert e over its contiguous 1024-token chunk):
    h = gelu(x_e @ w1[e] + b1[e]);  y_e = h @ w2[e] + b2[e]

Sharding: expert parallelism — core e holds expert e's weights and its token
chunk (the gate yields equal contiguous chunks, so no all-to-all is needed).
Each core runs the same SPMD program on its own data.

Per-core kernel (T=1024 tokens, D=1024, F=4096), all matmuls in fp16 with
fp32 PSUM accumulation (215.8 ns per 512-wide matmul — the PE's warm floor;
fp32 would be 4x slower, fp8 fails the accuracy gate):
  phase 1: for each 128-wide f-tile: h^T[ft] = gelu(w1[:,ft]^T @ x^T + b1[ft])
           (f on partitions -> b1 is a per-partition ACT bias; h^T resident in SBUF)
  phase 2: for each 128-wide dm-tile: y^T[dmo] = w2[:,dmo]^T @ h^T + b2[dmo]
           (dm-tile outer -> one 2-bank PSUM accumulator at a time)
All layout transposes/repacks are done on the host so every DMA is a large
partition-contiguous stream.

Head: descriptor issue costs ~0.65us/DMA per queue and doorbell-to-data is
~2us, so the critical first tiles (w1[0..4], x chunk 0 in quarters, x chunk 1
in halves, biases) are issued in parallel across the three DMA-capable queues
(Sync, Scalar, GpSimd) right at kernel start. A short burst of dummy matmuls
on memset scratch bridges the PE from kernel start to first-data and begins
warming the HAM clock gate; the first real matmul groups then run while the
clock finishes warming. Tail: the last dm-tile accumulates in 512/256/256
column chunks so the final flush (bias-add + store) is short, with the last
two stores issued on different queues so their descriptor writes overlap.
"""

import os

import numpy as np

# The kernel executes through the axon PJRT backend; a CPU pin (e.g. set for
# a jax reference run) would break NEFF dispatch in this process.
if os.environ.get("JAX_PLATFORMS") == "cpu":
    del os.environ["JAX_PLATFORMS"]

E = 8          # experts == cores
B, S = 2, 4096
D = 1024       # d_model
F = 4096       # d_ff
T = (B * S) // E  # tokens per expert chunk = 1024
P = 128
DO = D // P    # 8  k-tiles of d_model
FT = F // P    # 32 f-tiles of d_ff
DMO = D // P   # 8  output dm-tiles
FT2 = FT // 2  # half-slab of w2 f-tiles
NCHUNK = T // 512  # 2 moving-operand chunks (PSUM bank caps matmul N at 512)
N_WARMUP_MM = 7
N_PRELOAD = 5  # w1 tiles issued during the head DMA burst

_cached = None


def _build():
    import concourse.mybir as mybir
    import concourse.tile as tile
    from concourse import bacc
    from concourse.tile_rust import add_dep_helper

    f32 = mybir.dt.float32
    f16 = mybir.dt.float16

    nc = bacc.Bacc("TRN2", target_bir_lowering=False, debug=False, num_devices=E)

    xT_d = nc.dram_tensor("xT", [NCHUNK, P, DO, 512], f16, kind="ExternalInput")
    w1_d = nc.dram_tensor("w1r", [FT, P, DO, P], f16, kind="ExternalInput")
    bc_d = nc.dram_tensor("bc", [P, FT + DMO], f32, kind="ExternalInput")
    w2_d = nc.dram_tensor("w2r", [DMO, 2, P, FT2, P], f16, kind="ExternalInput")
    yT_d = nc.dram_tensor("yT", [DMO, P, T], f32, kind="ExternalOutput")

    gelu = mybir.ActivationFunctionType.Gelu_apprx_tanh

    with tile.TileContext(nc) as tc:
        with (
            tc.tile_pool(name="xpool", bufs=1) as xpool,
            tc.tile_pool(name="hpool", bufs=1) as hpool,
            tc.tile_pool(name="wpool", bufs=2) as wpool,
            tc.tile_pool(name="cpool", bufs=1) as cpool,
            tc.tile_pool(name="ypool", bufs=2) as ypool,
            tc.tile_pool(name="psum_h", bufs=2, space="PSUM") as psum_h,
            tc.tile_pool(name="psum_y", bufs=2, space="PSUM") as psum_y,
        ):
            # ---- head: queue the critical input DMAs across all three
            # DMA-capable queues so descriptor issue (~0.65us each, serial
            # per queue) and the ~2us doorbell-to-data latency overlap.
            w1_tiles = {}
            for ft in range(N_PRELOAD):
                w1_tiles[ft] = wpool.tile(
                    [P, DO, P], f16, tag="w1", bufs=7, name="w1_sb"
                )
            # contiguous-per-partition destination: [p, c, do*512]
            xT_sb = xpool.tile([P, NCHUNK, DO * 512], f16)
            bc_sb = cpool.tile([P, FT + DMO], f32)

            HX = DO * 512 // 2  # x chunk half (4 do-tiles)
            xc0 = xT_d.ap()[0].rearrange("p do t -> p (do t)")
            xc1 = xT_d.ap()[1].rearrange("p do t -> p (do t)")
            # Delivery on each HW queue is strictly serial at ~350 GB/s with
            # ~1us turnaround between descriptors, so split group 1's needs
            # across the two HW queues by consumption parity: Sync carries
            # w1[0] then x0's second half, Scalar (free until the first gelu)
            # carries x0's first half then w1[1]. x chunk 1 halves follow on
            # the same split; GpSimd (software DGE, slowest) gets the
            # latest-needed head tiles.
            nc.sync.dma_start(w1_tiles[0][:], w1_d.ap()[0])
            nc.scalar.dma_start(xT_sb[:, 0, :HX], xc0[:, :HX])
            nc.sync.dma_start(xT_sb[:, 0, HX:], xc0[:, HX:])
            nc.scalar.dma_start(w1_tiles[1][:], w1_d.ap()[1])
            nc.scalar.dma_start(xT_sb[:, 1, :HX], xc1[:, :HX])
            nc.sync.dma_start(xT_sb[:, 1, HX:], xc1[:, HX:])
            nc.sync.dma_start(w1_tiles[3][:], w1_d.ap()[3])
            nc.scalar.dma_start(w1_tiles[4][:], w1_d.ap()[4])
            nc.gpsimd.dma_start(w1_tiles[2][:], w1_d.ap()[2])
            nc.gpsimd.dma_start(bc_sb[:], bc_d.ap())
            b1_sb = bc_sb[:, :FT]
            b2_sb = bc_sb[:, FT:]

            # scratch for PE warmup: direct fp16 memset on the (otherwise
            # idle) vector engine — no fp32 staging, no cast
            scratch = cpool.tile([P, 512], f16)
            nc.vector.memset(scratch[:], 0.0)

            # PE warmup: dummy matmuls on scratch while the head DMAs are in
            # flight. Bridges kernel-start -> first-data and starts the HAM
            # clock-gate warming.
            for i in range(N_WARMUP_MM):
                pw = psum_y.tile([P, 512], f32, tag="py", name="pwarm")
                nc.tensor.matmul(
                    pw[:], scratch[:, :P], scratch[:], start=True, stop=True
                )

            h_sb = hpool.tile([P, FT, T], f16)

            # ---- phase 1: h^T = gelu(w1^T x^T + b1), one 128-row f-tile at a time
            # per-(ft, chunk) 1-bank PSUM tiles; the first three f-tiles run
            # chunk-0 first so the PE streams while x chunk 1 is on the wire
            def mm1_group(ph, w1_sb, c):
                mm = None
                for do in range(DO):
                    mm = nc.tensor.matmul(
                        ph[:],
                        w1_sb[:, do, :],
                        xT_sb[:, c, do * 512 : (do + 1) * 512],
                        start=(do == 0),
                        stop=(do == DO - 1),
                    )
                return mm

            def gelu_chunk(ph, ft, c):
                cs = slice(c * 512, (c + 1) * 512)
                return nc.scalar.activation(
                    h_sb[:, ft, cs], ph[:], gelu, bias=b1_sb[:, ft : ft + 1]
                )

            gelu_insts = {}
            HEAD = 3
            head_ph = {}
            for ft in range(HEAD):
                ph = psum_h.tile([P, 512], f32, tag="ph", bufs=4, name="ph")
                head_ph[ft] = ph
                mm1_group(ph, w1_tiles[ft], 0)
            for ft in range(HEAD):
                ph = head_ph[ft]
                gelu_insts[(ft, 0)] = gelu_chunk(ph, ft, 0)
                ph2 = psum_h.tile([P, 512], f32, tag="ph", bufs=4, name="ph")
                mm1_group(ph2, w1_tiles[ft], 1)
                gelu_insts[(ft, 1)] = gelu_chunk(ph2, ft, 1)

            for ft in range(HEAD, FT):
                if ft >= N_PRELOAD:
                    w1_tiles[ft] = wpool.tile(
                        [P, DO, P], f16, tag="w1", bufs=7, name="w1_sb"
                    )
                    nc.sync.dma_start(w1_tiles[ft][:], w1_d.ap()[ft])
                w1_sb = w1_tiles[ft]
                for c in range(NCHUNK):
                    ph = psum_h.tile([P, 512], f32, tag="ph", bufs=4, name="ph")
                    mm1_group(ph, w1_sb, c)
                    gelu_insts[(ft, c)] = gelu_chunk(ph, ft, c)

            # ---- phase 2: y^T[dmo] = w2[:,dmo]^T h^T + b2[dmo]
            FQ = FT // 4
            for dmo in range(DMO):
                w2_q = []
                for qq in range(4):
                    w2_sb = wpool.tile([P, FQ, P], f16, tag="w2", bufs=32, name="w2_sb")
                    dma = nc.sync.dma_start(
                        w2_sb[:],
                        w2_d.ap()[dmo, qq // 2, :, (qq % 2) * FQ : (qq % 2 + 1) * FQ],
                    )
                    if dmo == 0:
                        # keep dmo 0's prefetch out of the head's w1/xT window
                        add_dep_helper(
                            dma.ins,
                            gelu_insts[(6, 1)].ins,
                            sync=True,
                            reason="delay w2 prefetch past the kernel head",
                        )
                    w2_q.append(w2_sb)

                def y_flush(py_ap, cs, engine):
                    # bias-add in 256-col chunks so stores overlap the adds
                    y_sb = ypool.tile([P, 256], f32, tag="y", bufs=4, name="y_sb")
                    nc.vector.tensor_scalar_add(
                        y_sb[:], py_ap, b2_sb[:, dmo : dmo + 1]
                    )
                    engine.dma_start(yT_d.ap()[dmo, :, cs], y_sb[:])

                if dmo < DMO - 1:
                    py = psum_y.tile([P, T], f32, tag="py", name="py")
                    for fo in range(FT):
                        wt = w2_q[fo // FQ][:, fo % FQ, :]
                        for c in range(NCHUNK):
                            cs = slice(c * 512, (c + 1) * 512)
                            nc.tensor.matmul(
                                py[:, cs],
                                wt,
                                h_sb[:, fo, cs],
                                start=(fo == 0),
                                stop=(fo == FT - 1),
                            )
                    for c in range(4):
                        cs = slice(c * 256, (c + 1) * 256)
                        y_flush(py[:, cs], cs, nc.sync)
                else:
                    # last dm-tile: accumulate in 512/256/256-column chunks so
                    # only one short bias-add + store trails the final matmul.
                    # Separate PSUM tiles per chunk so Tile doesn't serialize
                    # one chunk's reads against the next chunk's writes. The
                    # final two stores go out on different queues so their
                    # ~0.6us descriptor writes overlap.
                    chunks = [(0, 512), (512, 256), (768, 256)]
                    for ci, (c0, cw) in enumerate(chunks):
                        py_c = psum_y.tile([P, cw], f32, tag="py", name="py_c")
                        for fo in range(FT):
                            wt = w2_q[fo // FQ][:, fo % FQ, :]
                            nc.tensor.matmul(
                                py_c[:],
                                wt,
                                h_sb[:, fo, c0 : c0 + cw],
                                start=(fo == 0),
                                stop=(fo == FT - 1),
                            )
                        for cc in range(cw // 256):
                            cs = slice(c0 + cc * 256, c0 + (cc + 1) * 256)
                            last = ci == len(chunks) - 1
                            y_flush(
                                py_c[:, cc * 256 : (cc + 1) * 256],
                                cs,
                                nc.scalar if last else nc.sync,
                            )

    nc.compile()
    return nc


def _get_nc():
    global _cached
    if _cached is None:
        _cached = _build()
    return _cached


def make_in_maps(x, w1, b1, w2, b2):
    x = np.asarray(x, dtype=np.float32)
    w1 = np.asarray(w1, dtype=np.float32)
    b1 = np.asarray(b1, dtype=np.float32)
    w2 = np.asarray(w2, dtype=np.float32)
    b2 = np.asarray(b2, dtype=np.float32)

    tokens = x.reshape(E, T, D)
    in_maps = []
    for e in range(E):
        xT = np.ascontiguousarray(
            tokens[e].reshape(NCHUNK, 512, DO, P).transpose(0, 3, 2, 1)
        ).astype(np.float16)  # [c, p, do, t']
        w1r = np.ascontiguousarray(
            w1[e].reshape(DO, P, FT, P).transpose(2, 1, 0, 3)
        ).astype(np.float16)  # [ft, p, do, j]
        bc = np.ascontiguousarray(
            np.concatenate([b1[e].reshape(FT, P).T, b2[e].reshape(DMO, P).T], axis=1)
        )  # [p, ft..dmo]
        w2r = np.ascontiguousarray(
            w2[e].reshape(2, FT2, P, DMO, P).transpose(3, 0, 2, 1, 4)
        ).astype(np.float16)  # [dmo, half, p, fo, j]
        in_maps.append({"xT": xT, "w1r": w1r, "bc": bc, "w2r": w2r})
    return in_maps


def gather_out(results):
    out = np.empty((E, T, D), dtype=np.float32)
    for e in range(E):
        yT = results[e]["yT"]  # [dmo, p, t]
        out[e] = yT.transpose(2, 0, 1).reshape(T, D)
    return out.reshape(B, S, D)


def kernel(x, w1, b1, w2, b2):
    from concourse.bass_utils import run_bass_kernel_spmd

    nc = _get_nc()
    in_maps = make_in_maps(x, w1, b1, w2, b2)
    res = run_bass_kernel_spmd(nc, in_maps, core_ids=list(range(E)))
    return gather_out(res.results)


# revision 11
# speedup vs baseline: 1.0240x; 1.0110x over previous
"""Trainium2 Bass kernel for an 8-expert MoE FFN layer (nn_MoELayer).

Reference computation (per expert e over its contiguous 1024-token chunk):
    h = gelu(x_e @ w1[e] + b1[e]);  y_e = h @ w2[e] + b2[e]

Sharding: expert parallelism — core e holds expert e's weights and its token
chunk (the gate yields equal contiguous chunks, so no all-to-all is needed).
Each core runs the same SPMD program on its own data.

Per-core kernel (T=1024 tokens, D=1024, F=4096), all matmuls in fp16 with
fp32 PSUM accumulation (215.8 ns per 512-wide matmul — the PE's warm floor;
fp32 would be 4x slower, fp8 fails the accuracy gate):
  phase 1: for each 128-wide f-tile: h^T[ft] = gelu(w1[:,ft]^T @ x^T + b1[ft])
           (f on partitions -> b1 is a per-partition ACT bias; h^T resident in SBUF)
  phase 2: for each 128-wide dm-tile: y^T[dmo] = w2[:,dmo]^T @ h^T + b2[dmo]
           (dm-tile outer -> one 2-bank PSUM accumulator at a time)
All layout transposes/repacks are done on the host so every DMA is a large
partition-contiguous stream.

Head: descriptor issue costs ~0.65us/DMA per queue and doorbell-to-data is
~2us, so the critical first tiles (w1[0..4], x chunk 0 in quarters, x chunk 1
in halves, biases) are issued in parallel across the three DMA-capable queues
(Sync, Scalar, GpSimd) right at kernel start. A short burst of dummy matmuls
on memset scratch bridges the PE from kernel start to first-data and begins
warming the HAM clock gate; the first real matmul groups then run while the
clock finishes warming. Tail: the last dm-tile accumulates in 512/256/256
column chunks so the final flush (bias-add + store) is short, with the last
two stores issued on different queues so their descriptor writes overlap.
"""

import os

import numpy as np

# The kernel executes through the axon PJRT backend; a CPU pin (e.g. set for
# a jax reference run) would break NEFF dispatch in this process.
if os.environ.get("JAX_PLATFORMS") == "cpu":
    del os.environ["JAX_PLATFORMS"]

E = 8          # experts == cores
B, S = 2, 4096
D = 1024       # d_model
F = 4096       # d_ff
T = (B * S) // E  # tokens per expert chunk = 1024
P = 128
DO = D // P    # 8  k-tiles of d_model
FT = F // P    # 32 f-tiles of d_ff
DMO = D // P   # 8  output dm-tiles
FT2 = FT // 2  # half-slab of w2 f-tiles
NCHUNK = T // 512  # 2 moving-operand chunks (PSUM bank caps matmul N at 512)
N_WARMUP_MM = 12
N_PRELOAD = 5  # w1 tiles resident before the ft loop takes over

_cached = None


def _build():
    import concourse.mybir as mybir
    import concourse.tile as tile
    from concourse import bacc
    from concourse.tile_rust import add_dep_helper

    f32 = mybir.dt.float32
    f16 = mybir.dt.float16

    nc = bacc.Bacc("TRN2", target_bir_lowering=False, debug=False, num_devices=E)

    # head blob: w1[0] followed by x chunk 0, packed so group 1's entire
    # working set streams in as ONE descriptor (serial-queue delivery has
    # ~1us dead time between descriptors; one blob avoids it)
    HB = DO * P + DO * 512  # 5120 fp16 per partition
    head_d = nc.dram_tensor("head", [P, HB], f16, kind="ExternalInput")
    xT_d = nc.dram_tensor("xT", [NCHUNK, P, DO, 512], f16, kind="ExternalInput")
    w1_d = nc.dram_tensor("w1r", [FT, P, DO, P], f16, kind="ExternalInput")
    bc_d = nc.dram_tensor("bc", [P, FT + DMO], f32, kind="ExternalInput")
    w2_d = nc.dram_tensor("w2r", [DMO, 2, P, FT2, P], f16, kind="ExternalInput")
    yT_d = nc.dram_tensor("yT", [DMO, P, T], f32, kind="ExternalOutput")

    gelu = mybir.ActivationFunctionType.Gelu_apprx_tanh

    with tile.TileContext(nc) as tc:
        with (
            tc.tile_pool(name="xpool", bufs=1) as xpool,
            tc.tile_pool(name="hpool", bufs=1) as hpool,
            tc.tile_pool(name="wpool", bufs=2) as wpool,
            tc.tile_pool(name="cpool", bufs=1) as cpool,
            tc.tile_pool(name="ypool", bufs=2) as ypool,
            tc.tile_pool(name="psum_h", bufs=2, space="PSUM") as psum_h,
            tc.tile_pool(name="psum_y", bufs=2, space="PSUM") as psum_y,
        ):
            # ---- head: queue the critical input DMAs across all three
            # DMA-capable queues so descriptor issue (~0.65us each, serial
            # per queue) and the ~2us doorbell-to-data latency overlap.
            # head blob in one flat tile: [w1[0] | x chunk 0]
            head_sb = xpool.tile([P, HB], f16)
            x1_sb = xpool.tile([P, DO * 512], f16)  # x chunk 1
            bc_sb = cpool.tile([P, FT + DMO], f32)
            w1_tiles = {}
            for ft in range(1, N_PRELOAD):
                w1_tiles[ft] = wpool.tile(
                    [P, DO, P], f16, tag="w1", bufs=7, name="w1_sb"
                )

            xc1 = xT_d.ap()[1].rearrange("p do t -> p (do t)")
            # Sync (fastest to first-data): the group-1 blob, then w1[3].
            # Scalar (free until the first gelu): w1[1], then x chunk 1.
            # GpSimd (software DGE, slowest): w1[2], biases, w1[4].
            nc.sync.dma_start(head_sb[:], head_d.ap())
            nc.scalar.dma_start(w1_tiles[1][:], w1_d.ap()[1])
            nc.scalar.dma_start(x1_sb[:], xc1)
            nc.sync.dma_start(w1_tiles[3][:], w1_d.ap()[3])
            nc.gpsimd.dma_start(w1_tiles[2][:], w1_d.ap()[2])
            nc.gpsimd.dma_start(bc_sb[:], bc_d.ap())
            nc.gpsimd.dma_start(w1_tiles[4][:], w1_d.ap()[4])
            b1_sb = bc_sb[:, :FT]
            b2_sb = bc_sb[:, FT:]

            # scratch for PE warmup: direct fp16 memset on the (otherwise
            # idle) vector engine — no fp32 staging, no cast
            scratch = cpool.tile([P, 512], f16)
            nc.vector.memset(scratch[:], 0.0)

            # PE warmup: dummy matmuls on scratch while the head DMAs are in
            # flight. Bridges kernel-start -> first-data and starts the HAM
            # clock-gate warming.
            for i in range(N_WARMUP_MM):
                pw = psum_y.tile([P, 512], f32, tag="py", name="pwarm")
                nc.tensor.matmul(
                    pw[:], scratch[:, :P], scratch[:], start=True, stop=True
                )

            h_sb = hpool.tile([P, FT, T], f16)

            # ---- phase 1: h^T = gelu(w1^T x^T + b1), one 128-row f-tile at a time
            # per-(ft, chunk) 1-bank PSUM tiles; the first three f-tiles run
            # chunk-0 first so the PE streams while x chunk 1 is on the wire
            XOFF = DO * P  # x chunk 0 offset inside the head blob

            def x_slice(c, do):
                if c == 0:
                    return head_sb[:, XOFF + do * 512 : XOFF + (do + 1) * 512]
                return x1_sb[:, do * 512 : (do + 1) * 512]

            def w1_slice(ft, do):
                if ft == 0:
                    return head_sb[:, do * P : (do + 1) * P]
                return w1_tiles[ft][:, do, :]

            def mm1_group(ph, ft, c):
                mm = None
                for do in range(DO):
                    mm = nc.tensor.matmul(
                        ph[:],
                        w1_slice(ft, do),
                        x_slice(c, do),
                        start=(do == 0),
                        stop=(do == DO - 1),
                    )
                return mm

            def gelu_chunk(ph, ft, c):
                cs = slice(c * 512, (c + 1) * 512)
                return nc.scalar.activation(
                    h_sb[:, ft, cs], ph[:], gelu, bias=b1_sb[:, ft : ft + 1]
                )

            gelu_insts = {}
            HEAD = 3
            head_ph = {}
            for ft in range(HEAD):
                ph = psum_h.tile([P, 512], f32, tag="ph", bufs=4, name="ph")
                head_ph[ft] = ph
                mm1_group(ph, ft, 0)
            for ft in range(HEAD):
                ph = head_ph[ft]
                gelu_insts[(ft, 0)] = gelu_chunk(ph, ft, 0)
                ph2 = psum_h.tile([P, 512], f32, tag="ph", bufs=4, name="ph")
                mm1_group(ph2, ft, 1)
                gelu_insts[(ft, 1)] = gelu_chunk(ph2, ft, 1)

            for ft in range(HEAD, FT):
                if ft >= N_PRELOAD:
                    w1_tiles[ft] = wpool.tile(
                        [P, DO, P], f16, tag="w1", bufs=7, name="w1_sb"
                    )
                    nc.sync.dma_start(w1_tiles[ft][:], w1_d.ap()[ft])
                for c in range(NCHUNK):
                    ph = psum_h.tile([P, 512], f32, tag="ph", bufs=4, name="ph")
                    mm1_group(ph, ft, c)
                    gelu_insts[(ft, c)] = gelu_chunk(ph, ft, c)

            # ---- phase 2: y^T[dmo] = w2[:,dmo]^T h^T + b2[dmo]
            FQ = FT // 4
            for dmo in range(DMO):
                w2_q = []
                for qq in range(4):
                    w2_sb = wpool.tile([P, FQ, P], f16, tag="w2", bufs=32, name="w2_sb")
                    dma = nc.sync.dma_start(
                        w2_sb[:],
                        w2_d.ap()[dmo, qq // 2, :, (qq % 2) * FQ : (qq % 2 + 1) * FQ],
                    )
                    if dmo == 0:
                        # keep dmo 0's prefetch out of the head's w1/xT window
                        add_dep_helper(
                            dma.ins,
                            gelu_insts[(6, 1)].ins,
                            sync=True,
                            reason="delay w2 prefetch past the kernel head",
                        )
                    w2_q.append(w2_sb)

                def y_flush(py_ap, cs, engine):
                    # bias-add in 256-col chunks so stores overlap the adds
                    y_sb = ypool.tile([P, 256], f32, tag="y", bufs=4, name="y_sb")
                    nc.vector.tensor_scalar_add(
                        y_sb[:], py_ap, b2_sb[:, dmo : dmo + 1]
                    )
                    engine.dma_start(yT_d.ap()[dmo, :, cs], y_sb[:])

                if dmo < DMO - 1:
                    py = psum_y.tile([P, T], f32, tag="py", name="py")
                    for fo in range(FT):
                        wt = w2_q[fo // FQ][:, fo % FQ, :]
                        for c in range(NCHUNK):
                            cs = slice(c * 512, (c + 1) * 512)
                            nc.tensor.matmul(
                                py[:, cs],
                                wt,
                                h_sb[:, fo, cs],
                                start=(fo == 0),
                                stop=(fo == FT - 1),
                            )
                    for c in range(4):
                        cs = slice(c * 256, (c + 1) * 256)
                        y_flush(py[:, cs], cs, nc.sync)
                else:
                    # last dm-tile: accumulate in 512/256/256-column chunks so
                    # only one short bias-add + store trails the final matmul.
                    # Separate PSUM tiles per chunk so Tile doesn't serialize
                    # one chunk's reads against the next chunk's writes. The
                    # final two stores go out on different queues so their
                    # ~0.6us descriptor writes overlap.
                    chunks = [(0, 512), (512, 256), (768, 256)]
                    for ci, (c0, cw) in enumerate(chunks):
                        py_c = psum_y.tile([P, cw], f32, tag="py", name="py_c")
                        for fo in range(FT):
                            wt = w2_q[fo // FQ][:, fo % FQ, :]
                            nc.tensor.matmul(
                                py_c[:],
                                wt,
                                h_sb[:, fo, c0 : c0 + cw],
                                start=(fo == 0),
                                stop=(fo == FT - 1),
                            )
                        for cc in range(cw // 256):
                            cs = slice(c0 + cc * 256, c0 + (cc + 1) * 256)
                            last = ci == len(chunks) - 1
                            y_flush(
                                py_c[:, cc * 256 : (cc + 1) * 256],
                                cs,
                                nc.scalar if last else nc.sync,
                            )

    nc.compile()
    return nc


def _get_nc():
    global _cached
    if _cached is None:
        _cached = _build()
    return _cached


def make_in_maps(x, w1, b1, w2, b2):
    x = np.asarray(x, dtype=np.float32)
    w1 = np.asarray(w1, dtype=np.float32)
    b1 = np.asarray(b1, dtype=np.float32)
    w2 = np.asarray(w2, dtype=np.float32)
    b2 = np.asarray(b2, dtype=np.float32)

    tokens = x.reshape(E, T, D)
    in_maps = []
    for e in range(E):
        xT = np.ascontiguousarray(
            tokens[e].reshape(NCHUNK, 512, DO, P).transpose(0, 3, 2, 1)
        ).astype(np.float16)  # [c, p, do, t']
        w1r = np.ascontiguousarray(
            w1[e].reshape(DO, P, FT, P).transpose(2, 1, 0, 3)
        ).astype(np.float16)  # [ft, p, do, j]
        # head blob: per partition [w1[0] (do-major) | x chunk 0 (do-major)]
        head = np.concatenate(
            [
                w1r[0].reshape(P, DO * P),
                xT[0].reshape(P, DO * 512),
            ],
            axis=1,
        )  # [p, 5120]
        bc = np.ascontiguousarray(
            np.concatenate([b1[e].reshape(FT, P).T, b2[e].reshape(DMO, P).T], axis=1)
        )  # [p, ft..dmo]
        w2r = np.ascontiguousarray(
            w2[e].reshape(2, FT2, P, DMO, P).transpose(3, 0, 2, 1, 4)
        ).astype(np.float16)  # [dmo, half, p, fo, j]
        in_maps.append(
            {"head": head, "xT": xT, "w1r": w1r, "bc": bc, "w2r": w2r}
        )
    return in_maps


def gather_out(results):
    out = np.empty((E, T, D), dtype=np.float32)
    for e in range(E):
        yT = results[e]["yT"]  # [dmo, p, t]
        out[e] = yT.transpose(2, 0, 1).reshape(T, D)
    return out.reshape(B, S, D)


def kernel(x, w1, b1, w2, b2):
    from concourse.bass_utils import run_bass_kernel_spmd

    nc = _get_nc()
    in_maps = make_in_maps(x, w1, b1, w2, b2)
    res = run_bass_kernel_spmd(nc, in_maps, core_ids=list(range(E)))
    return gather_out(res.results)
